# revision 1
# baseline (speedup 1.0000x reference)
"""Trainium2 Bass kernel for nn_LinearAttention (random-feature attention).

Reference computation (B=4, S=4096, D=U=R=256, fp32):
    Q = query @ Wq + bq                      [B,S,U]
    K = value @ Wk + bk                      [B,S,U]
    V = value @ Wv + bv                      [B,S,U]
    K_hat = cos(K @ Wr + br)                 [B,S,R]
    out = softmax(Q @ K_hat^T) @ V           [B,S,U]

Sharding: 8 cores, core c handles batch b=c//2, query-half h=c%2 (2048
queries). Each core needs the full key/value sequence of its batch.

Per-core layout strategy (feature-on-partitions so the whole matmul chain
runs without intermediate transposes):
    query^T, value^T via PE transpose (fp32 has no DMA-transpose path)
    Q^T[u,q]    = Wq.T @ query^T      (+bq per-partition)
    K^T[u,s]    = Wk.T @ value^T      (+bk per-partition)
    K_hat^T[r,s]= cos(Wr.T @ K^T + br)   via exact range reduction
    V[s,u]      = (value @ Wv) + ones-row x bv   (natural layout)
    scores^T[k,q] = K_hat^T_chunk.T @ Q^T        (PSUM, 2 r-chunks)
    probs^T = exp(scores^T)           (no max-subtraction: |scores| < ~70,
                                       HW exp is accurate to +-87)
    rowsum partials on DVE+GPSIMD, 128->1 reduction via tiny N=1 matmuls
    out^T[u,q] += V_chunk.T @ probs^T (PSUM accumulate over 32 k-chunks)
    out = transpose(out^T) * recip(rowsum)  -> DMA

All high-volume matmuls use float32r operands (TF32-class, 1 cycle/row on
the PE vs 4 for fp32 — measured 1.5e-4 rel err on a 256-deep dot product).
Producers round to float32r on-device (walrus verifier requires it).
"""
import sys

if "/opt/trn_rl_repo" not in sys.path:
    sys.path.insert(0, "/opt/trn_rl_repo")

import numpy as np
import concourse.bass as bass
import concourse.bacc as bacc
import concourse.tile as tile
from concourse import mybir
from concourse.bass_utils import run_bass_kernel_spmd
from concourse.masks import make_identity

FP = mybir.dt.float32
FR = mybir.dt.float32r
AF = mybir.ActivationFunctionType

P = 128          # partitions
B, S, DIM = 4, 4096, 256
SQ = S // 2      # queries per core
NC = 8           # cores
DC = DIM // P    # 2 chunks of the feature dims (d, u, r)
KT = S // P      # 32 key chunks
QB = 512         # q-block (psum bank = 512 fp32)
NQB = SQ // QB   # 4 q-blocks
ST = S // P      # 32 seq tiles for value
QT = SQ // P     # 16 seq tiles for query
TPB = QB // P    # seq tiles per block

INV2PI = float(1.0 / (2.0 * np.pi))
MAGIC = 12582912.0  # 1.5 * 2^23: fp32 round-to-nearest-int trick


def build_kernel(nc: bass.Bass):
    ADD, SUB, MUL = (mybir.AluOpType.add, mybir.AluOpType.subtract,
                     mybir.AluOpType.mult)
    q_in = nc.dram_tensor("q_shard", [SQ, DIM], FP, kind="ExternalInput")
    v_in = nc.dram_tensor("v_full", [S, DIM], FP, kind="ExternalInput")
    w_q = nc.dram_tensor("Wq", [DIM, DIM], FP, kind="ExternalInput")
    w_k = nc.dram_tensor("Wk", [DIM, DIM], FP, kind="ExternalInput")
    w_v = nc.dram_tensor("Wv", [DIM, DIM], FP, kind="ExternalInput")
    w_r = nc.dram_tensor("Wr", [DIM, DIM], FP, kind="ExternalInput")
    b_q = nc.dram_tensor("bq", [DIM], FP, kind="ExternalInput")
    b_k = nc.dram_tensor("bk", [DIM], FP, kind="ExternalInput")
    b_v = nc.dram_tensor("bv", [DIM], FP, kind="ExternalInput")
    b_r = nc.dram_tensor("br", [DIM], FP, kind="ExternalInput")
    out = nc.dram_tensor("out", [SQ, DIM], FP, kind="ExternalOutput")

    with tile.TileContext(nc) as tc:
        with tc.tile_pool(name="singles", bufs=1) as singles, \
             tc.tile_pool(name="persist", bufs=1) as persist:
            ident = singles.tile([P, P], FP)
            make_identity(nc, ident)
            ones_col = singles.tile([P, 1], FP)
            nc.vector.memset(ones_col, 1.0)
            ones_row_st = singles.tile([1, P], FP)
            nc.vector.memset(ones_row_st, 1.0)
            ones_row = singles.tile([1, P], FR)
            nc.vector.tensor_copy(ones_row, ones_row_st)

            # weights: DMA fp32 staging -> DVE round-copy to float32r
            w_fr = {}
            for name, dram in (("wq", w_q), ("wk", w_k), ("wv", w_v),
                               ("wr", w_r)):
                stage = singles.tile([P, DC, DIM], FP, tag=f"{name}_st")
                nc.sync.dma_start(out=stage,
                                  in_=dram.rearrange("(c p) u -> p c u", p=P))
                wt = singles.tile([P, DC, DIM], FR, tag=f"{name}_fr")
                nc.vector.tensor_copy(wt, stage)
                w_fr[name] = wt
            wq_sb, wk_sb, wv_sb, wr_sb = (w_fr["wq"], w_fr["wk"],
                                          w_fr["wv"], w_fr["wr"])
            # biases as [p, chunk]
            bq_sb = singles.tile([P, DC], FP)
            nc.sync.dma_start(out=bq_sb, in_=b_q.rearrange("(c p) -> p c", p=P))
            bk_sb = singles.tile([P, DC], FP)
            nc.sync.dma_start(out=bk_sb, in_=b_k.rearrange("(c p) -> p c", p=P))
            brs_sb = singles.tile([P, DC], FP)
            nc.sync.dma_start(out=brs_sb, in_=b_r.rearrange("(c p) -> p c", p=P))
            bv_stage = singles.tile([1, DIM], FP)
            nc.sync.dma_start(out=bv_stage,
                              in_=b_v.rearrange("(c u) -> c u", c=1))
            bv_row = singles.tile([1, DIM], FR)
            nc.vector.tensor_copy(bv_row, bv_stage)

            # persistent stage outputs (all float32r: consumed by matmuls)
            qT_p = persist.tile([P, DC, SQ], FR, tag="qT_proj")    # Q^T
            kh_sb = persist.tile([P, DC, S], FR, tag="khat")       # K_hat^T
            v_sb = persist.tile([P, ST, DIM], FR, tag="v_nat")     # V [k,u]

            # ---------------- stage B+C: transposes + projections -------------
            # Streamed per 512-seq block to bound SBUF: transpose 4 input
            # tiles, then immediately compute K^T, V, K_hat for that block.
            with tc.tile_pool(name="trans_in", bufs=8) as tin, \
                 tc.tile_pool(name="khtmp", bufs=2) as khtmp, \
                 tc.tile_pool(name="blocks", bufs=2) as blocks, \
                 tc.tile_pool(name="trans_ps", bufs=4, space="PSUM") as tps, \
                 tc.tile_pool(name="proj_ps", bufs=2, space="PSUM") as pps:
                def do_qb(qb):
                    # Q^T = Wq.T @ query^T + bq for one 512-query block
                    qT_blk = blocks.tile([P, DC, QB], FR, tag="qT_blk")
                    for st4 in range(TPB):
                        s0 = qb * QB + st4 * P
                        for dc in range(DC):
                            tmp = tin.tile([P, P], FP, tag="tr_tmp")
                            nc.sync.dma_start(
                                out=tmp,
                                in_=q_in[s0:s0 + P, dc * P:(dc + 1) * P])
                            tp = tps.tile([P, P], FP, tag="tr_ps")
                            nc.tensor.transpose(tp, tmp, ident)
                            if (st4 + dc) % 2 == 0:
                                nc.vector.tensor_copy(
                                    qT_blk[:, dc, st4 * P:(st4 + 1) * P], tp)
                            else:
                                nc.scalar.copy(
                                    qT_blk[:, dc, st4 * P:(st4 + 1) * P], tp)
                    for uc in range(DC):
                        ps = pps.tile([P, QB], FP, tag="proj")
                        for dc in range(DC):
                            nc.tensor.matmul(
                                ps, wq_sb[:, dc, uc * P:(uc + 1) * P],
                                qT_blk[:, dc, :],
                                start=(dc == 0), stop=(dc == DC - 1))
                        nc.vector.tensor_scalar_add(
                            qT_p[:, uc, qb * QB:(qb + 1) * QB], ps,
                            bq_sb[:, uc:uc + 1])

                for kb in range(S // QB):
                    if kb < NQB:
                        do_qb(kb)
                    vT_blk = blocks.tile([P, DC, QB], FR, tag="vT_blk")
                    for st4 in range(TPB):
                        s0 = kb * QB + st4 * P
                        for dc in range(DC):
                            tmp = tin.tile([P, P], FP, tag="tr_tmp")
                            nc.sync.dma_start(
                                out=tmp,
                                in_=v_in[s0:s0 + P, dc * P:(dc + 1) * P])
                            tp = tps.tile([P, P], FP, tag="tr_ps")
                            nc.tensor.transpose(tp, tmp, ident)
                            if (st4 + dc) % 2 == 0:
                                nc.vector.tensor_copy(
                                    vT_blk[:, dc, st4 * P:(st4 + 1) * P], tp)
                            else:
                                nc.scalar.copy(
                                    vT_blk[:, dc, st4 * P:(st4 + 1) * P], tp)

                    # V block (natural layout): V = value @ Wv + bv
                    for st4 in range(TPB):
                        ps = pps.tile([P, DIM], FP, tag="projv")
                        for dc in range(DC):
                            nc.tensor.matmul(
                                ps, vT_blk[:, dc, st4 * P:(st4 + 1) * P],
                                wv_sb[:, dc, :], start=(dc == 0), stop=False)
                        nc.tensor.matmul(ps, ones_row, bv_row,
                                         start=False, stop=True)
                        nc.scalar.copy(v_sb[:, kb * TPB + st4, :], ps)

                    # K^T block = Wk.T @ value^T + bk  (DVE rounds to fp32r)
                    kT_blk = blocks.tile([P, DC, QB], FR, tag="kT_blk")
                    for uc in range(DC):
                        ps = pps.tile([P, QB], FP, tag="proj")
                        for dc in range(DC):
                            nc.tensor.matmul(
                                ps, wk_sb[:, dc, uc * P:(uc + 1) * P],
                                vT_blk[:, dc, :],
                                start=(dc == 0), stop=(dc == DC - 1))
                        nc.vector.tensor_scalar_add(
                            kT_blk[:, uc, :], ps, bk_sb[:, uc:uc + 1])

                    # K_hat^T block = cos(Wr.T @ K^T + br).  HW Sin is only
                    # valid on ~[-2.1, 2.1], so range-reduce exactly:
                    #   t = y + br;  f = frac(t/2pi) in [-.5,.5] (magic-round)
                    #   cos(t) = 1 - 2 sin^2(pi f)   (sign of sin irrelevant)
                    for rc in range(DC):
                        ps = pps.tile([P, QB], FP, tag="proj")
                        for uc in range(DC):
                            nc.tensor.matmul(
                                ps, wr_sb[:, uc, rc * P:(rc + 1) * P],
                                kT_blk[:, uc, :],
                                start=(uc == 0), stop=(uc == DC - 1))
                        sl = slice(kb * QB, (kb + 1) * QB)
                        r_t = khtmp.tile([P, QB], FP, tag="kh_r")
                        nc.vector.tensor_scalar(
                            r_t, ps, brs_sb[:, rc:rc + 1], INV2PI, ADD, MUL)
                        m_t = khtmp.tile([P, QB], FP, tag="kh_m")
                        nc.gpsimd.tensor_scalar(m_t, r_t, MAGIC, MAGIC, ADD, SUB)
                        f_t = khtmp.tile([P, QB], FP, tag="kh_f")
                        nc.gpsimd.tensor_sub(f_t, r_t, m_t)
                        s_t = khtmp.tile([P, QB], FP, tag="kh_s")
                        nc.scalar.activation(s_t, f_t, AF.Sin,
                                             scale=float(np.pi))
                        q_t = khtmp.tile([P, QB], FP, tag="kh_q")
                        nc.scalar.activation(q_t, s_t, AF.Square,
                                             scale=float(np.sqrt(2.0)))
                        nc.vector.tensor_scalar(
                            kh_sb[:, rc, sl], q_t, -1.0, 1.0, MUL, ADD)

            # ---------------- stage D: attention ------------------------------
            with tc.tile_pool(name="attn", bufs=4) as attn, \
                 tc.tile_pool(name="accp", bufs=2) as accp, \
                 tc.tile_pool(name="outp", bufs=3) as outp, \
                 tc.tile_pool(name="sc_ps", bufs=2, space="PSUM") as scp, \
                 tc.tile_pool(name="pv_ps", bufs=2, space="PSUM") as pvp, \
                 tc.tile_pool(name="tr_ps2", bufs=2, space="PSUM") as trp:
                for qb in range(NQB):
                    qs = slice(qb * QB, (qb + 1) * QB)
                    # two independent rowsum-partial chains (DVE even kt,
                    # GPSIMD odd kt) so neither engine waits on the other;
                    # merged by the PSUM-accumulated rowsum matmuls below
                    acc0 = accp.tile([P, QB], FP, tag="acc0")
                    acc1 = accp.tile([P, QB], FP, tag="acc1")
                    pv0 = pvp.tile([P, QB], FP, tag="pv0")
                    pv1 = pvp.tile([P, QB], FP, tag="pv1")
                    pvs = (pv0, pv1)
                    for kt in range(KT):
                        sc = scp.tile([P, QB], FP, tag="sc")
                        for rc in range(DC):
                            nc.tensor.matmul(
                                sc, kh_sb[:, rc, kt * P:(kt + 1) * P],
                                qT_p[:, rc, qs],
                                start=(rc == 0), stop=(rc == DC - 1))
                        probs = attn.tile([P, QB], FR, tag="probs")
                        nc.scalar.activation(probs, sc, AF.Exp)
                        pf = probs.bitcast(FP)
                        if kt == 0:
                            nc.vector.tensor_copy(acc0, pf)
                        elif kt == 1:
                            nc.gpsimd.tensor_copy(acc1, pf)
                        elif kt % 2 == 0:
                            nc.vector.tensor_add(acc0, acc0, pf)
                        else:
                            nc.gpsimd.tensor_add(acc1, acc1, pf)
                        for uh in range(2):
                            nc.tensor.matmul(
                                pvs[uh], v_sb[:, kt, uh * P:(uh + 1) * P],
                                probs, start=(kt == 0), stop=(kt == KT - 1))

                    # rowsum 128->1, reciprocal, transpose-back, normalize, out
                    for qt in range(QB // P):
                        rs_t = trp.tile([P, P], FP, tag="ot_ps")
                        rs = rs_t[:, 0:1]
                        nc.tensor.matmul(
                            rs, acc0[:, qt * P:(qt + 1) * P], ones_col,
                            start=True, stop=False)
                        nc.tensor.matmul(
                            rs, acc1[:, qt * P:(qt + 1) * P], ones_col,
                            start=False, stop=True)
                        recip = outp.tile([P, 1], FP, tag="recip")
                        nc.vector.reciprocal(recip, rs)
                        o_sb = outp.tile([P, DIM], FP, tag="o_out")
                        for uh in range(2):
                            ot = outp.tile([P, P], FP, tag="ot")
                            nc.scalar.copy(
                                ot, pvs[uh][:, qt * P:(qt + 1) * P])
                            tp = trp.tile([P, P], FP, tag="ot_ps")
                            nc.tensor.transpose(tp, ot, ident)
                            nc.vector.tensor_scalar_mul(
                                o_sb[:, uh * P:(uh + 1) * P], tp, recip[:])
                        row0 = qb * QB + qt * P
                        nc.sync.dma_start(out=out[row0:row0 + P, :], in_=o_sb)
    nc.finalize()
    return nc


_NC_CACHE = None


def _get_nc():
    global _NC_CACHE
    if _NC_CACHE is None:
        _NC_CACHE = build_kernel(bacc.Bacc(None, target_bir_lowering=False))
    return _NC_CACHE


def kernel(**inputs) -> np.ndarray:
    query = np.ascontiguousarray(np.asarray(inputs["query"], dtype=np.float32))
    value = np.ascontiguousarray(np.asarray(inputs["value"], dtype=np.float32))
    ws = {k: np.ascontiguousarray(np.asarray(inputs[k], dtype=np.float32))
          for k in ("Wq", "bq", "Wk", "bk", "Wv", "bv", "Wr", "br")}
    nc = _get_nc()
    in_maps = []
    for c in range(NC):
        b, h = c // 2, c % 2
        in_maps.append({
            "q_shard": np.ascontiguousarray(query[b, h * SQ:(h + 1) * SQ]),
            "v_full": value[b],
            **ws,
        })
    res = run_bass_kernel_spmd(nc, in_maps, core_ids=list(range(NC)))
    out = np.empty((B, S, DIM), np.float32)
    for c in range(NC):
        b, h = c // 2, c % 2
        out[b, h * SQ:(h + 1) * SQ] = res.results[c]["out"]
    return out



# revision 9
# speedup vs baseline: 1.2135x; 1.2135x over previous
"""Trainium2 Bass kernel for nn_LinearAttention (random-feature attention).

Reference computation (B=4, S=4096, D=U=R=256, fp32):
    Q = query @ Wq + bq                      [B,S,U]
    K = value @ Wk + bk                      [B,S,U]
    V = value @ Wv + bv                      [B,S,U]
    K_hat = cos(K @ Wr + br)                 [B,S,R]
    out = softmax(Q @ K_hat^T) @ V           [B,S,U]

Sharding: 8 cores, core c handles batch b=c//2, query-half h=c%2 (2048
queries). Each core needs the full key/value sequence of its batch.

Per-core algebraic restructurings (all exact up to fp rounding):
  * W_kr = (Wk @ Wr)/2pi, b_kr = (Wr^T bk + br)/2pi precomputed on device:
    K_hat chain runs directly off value^T (K projection eliminated).
  * cos(t) = 1 - 2 sin^2(pi frac(t/2pi)) and softmax is invariant to a
    per-query shift, so we store kh := 2 sin^2(.) and negate Q: the
    "1 -" pass disappears and scores shrink (std 7 vs 12 - exp safe).
  * V projection deferred through the attention matmul (associativity):
    out^T = Wv^T (value^T probs^T) + rowsum * bv.  The PV stage uses raw
    value tiles as stationary operands (V projection + copies eliminated);
    a tiny per-q-block Wv^T @ Z fixup restores the projection; bv is added
    during the PSUM->SBUF move in the output stage (softmax rows sum to 1).
  * Transposes use a bf16 identity (cost follows the moving operand =
    identity: 1 cycle/row instead of 2) with float32r-bitcast data, which
    the simulator moves exactly.
  * All high-volume matmul moving operands are float32r (1 cycle/row).
    fp32 tiles are bitcast to float32r at the matmul call (no copies).
  * Attention kt-loop software-pipelined: PE stream is sc(kt), Z(kt-1) so
    the PE never waits on the Act exp; previous q-block's output drain is
    emitted into the next q-block's early iterations.
"""
import sys

if "/opt/trn_rl_repo" not in sys.path:
    sys.path.insert(0, "/opt/trn_rl_repo")

import numpy as np
import concourse.bass as bass
import concourse.bacc as bacc
import concourse.tile as tile
from concourse import mybir
from concourse.bass_utils import run_bass_kernel_spmd
from concourse.masks import make_identity

FP = mybir.dt.float32
FR = mybir.dt.float32r
BF = mybir.dt.bfloat16
AF = mybir.ActivationFunctionType

P = 128          # partitions
B, S, DIM = 4, 4096, 256
SQ = S // 2      # queries per core
NC = 8           # cores
DC = DIM // P    # 2 chunks of the feature dims (d, u, r)
KT = S // P      # 32 key tiles
QB = 512         # q-block (psum bank = 512 fp32)
NQB = SQ // QB   # 4 q-blocks
ST = S // P      # 32 seq tiles for value
TPB = QB // P    # seq tiles per 512-block

INV2PI = float(1.0 / (2.0 * np.pi))
MAGIC = 12582912.0  # 1.5 * 2^23: fp32 round-to-nearest-int trick
SQRT2 = float(np.sqrt(2.0))


def build_kernel(nc: bass.Bass):
    ADD, SUB, MUL = (mybir.AluOpType.add, mybir.AluOpType.subtract,
                     mybir.AluOpType.mult)
    q_in = nc.dram_tensor("q_shard", [SQ, DIM], FP, kind="ExternalInput")
    v_in = nc.dram_tensor("v_full", [S, DIM], FP, kind="ExternalInput")
    w_q = nc.dram_tensor("Wq", [DIM, DIM], FP, kind="ExternalInput")
    w_k = nc.dram_tensor("Wk", [DIM, DIM], FP, kind="ExternalInput")
    w_v = nc.dram_tensor("Wv", [DIM, DIM], FP, kind="ExternalInput")
    w_r = nc.dram_tensor("Wr", [DIM, DIM], FP, kind="ExternalInput")
    b_q = nc.dram_tensor("bq", [DIM], FP, kind="ExternalInput")
    b_k = nc.dram_tensor("bk", [DIM], FP, kind="ExternalInput")
    b_v = nc.dram_tensor("bv", [DIM], FP, kind="ExternalInput")
    b_r = nc.dram_tensor("br", [DIM], FP, kind="ExternalInput")
    out = nc.dram_tensor("out", [SQ, DIM], FP, kind="ExternalOutput")

    with tile.TileContext(nc) as tc:
        with tc.tile_pool(name="singles", bufs=1) as singles, \
             tc.tile_pool(name="persist", bufs=1) as persist:
            ident = singles.tile([P, P], BF)
            make_identity(nc, ident)
            ones_col = singles.tile([P, 1], FP)
            nc.vector.memset(ones_col, 1.0)
            ones_row = singles.tile([1, QB], FP)
            nc.vector.memset(ones_row, 1.0)

            # weights straight into fp32 SBUF; bitcast to float32r at use
            wq_sb = singles.tile([P, DC, DIM], FP)
            nc.sync.dma_start(out=wq_sb,
                              in_=w_q.rearrange("(c p) u -> p c u", p=P))
            wv_sb = singles.tile([P, DC, DIM], FP)
            nc.sync.dma_start(out=wv_sb,
                              in_=w_v.rearrange("(c p) u -> p c u", p=P))
            wr_sb = singles.tile([P, DC, DIM], FP)
            nc.sync.dma_start(out=wr_sb,
                              in_=w_r.rearrange("(c p) u -> p c u", p=P))
            bq_sb = singles.tile([P, DC], FP)
            nc.sync.dma_start(out=bq_sb, in_=b_q.rearrange("(c p) -> p c", p=P))
            bk_sb = singles.tile([P, DC], FP)
            nc.sync.dma_start(out=bk_sb, in_=b_k.rearrange("(c p) -> p c", p=P))
            br_sb = singles.tile([P, DC], FP)
            nc.sync.dma_start(out=br_sb, in_=b_r.rearrange("(c p) -> p c", p=P))
            bv_sb = singles.tile([P, DC], FP)
            nc.sync.dma_start(out=bv_sb, in_=b_v.rearrange("(c p) -> p c", p=P))

            wkr_sb = singles.tile([P, DC, DIM], FP)   # (Wk @ Wr) / 2pi
            bkr_row = singles.tile([1, DIM], FP)      # (Wr^T bk + br) / 2pi

            # persistent stage outputs
            qT_p = persist.tile([P, DC, SQ], FP, tag="qT_proj")  # -(Q^T)
            kh_sb = persist.tile([P, DC, S], FP, tag="khat")     # 2sin^2 form
            v_nat = persist.tile([P, ST, DIM], FP, tag="v_nat")  # raw value

            # ---- precompute W_kr, b_kr on device ------------------------
            with tc.tile_pool(name="pre", bufs=2) as pre, \
                 tc.tile_pool(name="pre_ps", bufs=2, space="PSUM") as preps:
                wk_st = pre.tile([P, DC, DIM], FP, tag="wk")
                nc.sync.dma_start(out=wk_st,
                                  in_=w_k.rearrange("(c p) u -> p c u", p=P))
                wkT = pre.tile([P, DC, DIM], FP, tag="wkT")  # [u, (dc d)]
                for uc in range(DC):
                    tb = preps.tile([P, DIM], FR, tag="tr")
                    for dc in range(DC):
                        nc.tensor.matmul(
                            tb[:, dc * P:(dc + 1) * P],
                            wk_st[:, dc, uc * P:(uc + 1) * P].bitcast(FR),
                            ident, is_transpose=True,
                            start=(dc == 0), stop=(dc == DC - 1))
                    nc.vector.tensor_copy(wkT[:, uc, :], tb)
                for dc in range(DC):
                    psw = preps.tile([P, DIM], FP, tag="wkr")
                    for uc in range(DC):
                        nc.tensor.matmul(
                            psw, wkT[:, uc, dc * P:(dc + 1) * P].bitcast(FR),
                            wr_sb[:, uc, :].bitcast(FR),
                            start=(uc == 0), stop=(uc == DC - 1))
                    nc.vector.tensor_scalar(wkr_sb[:, dc, :], psw,
                                            INV2PI, 0.0, MUL, ADD)
                bcol = pre.tile([P, DC], FR, tag="bcol")
                for rc in range(DC):
                    psb = preps.tile([P, 1], FP, tag="bkr")
                    for uc in range(DC):
                        nc.tensor.matmul(
                            psb, wr_sb[:, uc, rc * P:(rc + 1) * P].bitcast(FR),
                            bk_sb[:, uc:uc + 1].bitcast(FR),
                            start=(uc == 0), stop=(uc == DC - 1))
                    nc.vector.tensor_scalar(bcol[:, rc:rc + 1], psb,
                                            br_sb[:, rc:rc + 1], INV2PI,
                                            ADD, MUL)
                for rc in range(DC):
                    tbr = preps.tile([1, P], FR, tag="btr")
                    nc.tensor.matmul(tbr, bcol[:, rc:rc + 1], ident,
                                     is_transpose=True, start=True, stop=True)
                    nc.vector.tensor_copy(
                        bkr_row[0:1, rc * P:(rc + 1) * P], tbr)

            # ---- stage B/C: transposes, Q proj, K_hat -------------------
            copy_engines = (nc.vector, nc.scalar, nc.gpsimd)
            cp_i = 0
            with tc.tile_pool(name="qblk", bufs=2) as qblkp, \
                 tc.tile_pool(name="blocks", bufs=2) as blocks, \
                 tc.tile_pool(name="khtmp", bufs=2) as khtmp, \
                 tc.tile_pool(name="tps", bufs=2, space="PSUM") as tps, \
                 tc.tile_pool(name="pps", bufs=2, space="PSUM") as pps:

                def transpose_block(src, srow, blk_out):
                    # src[:, srow+t, dc*P:(dc+1)*P] tiles -> blk_out[:,dc,:]
                    nonlocal cp_i
                    for dc in range(DC):
                        bank = tps.tile([P, QB], FR, tag="tr")
                        for t in range(TPB):
                            nc.tensor.matmul(
                                bank[:, t * P:(t + 1) * P],
                                src[:, srow + t, dc * P:(dc + 1) * P].bitcast(FR),
                                ident, is_transpose=True,
                                start=(t == 0), stop=(t == TPB - 1))
                        eng = copy_engines[cp_i % 3]
                        cp_i += 1
                        if eng is nc.scalar:
                            eng.copy(blk_out[:, dc, :], bank)
                        else:
                            eng.tensor_copy(blk_out[:, dc, :], bank)

                def do_qb(qb):
                    qb_t = qblkp.tile([P, TPB, DIM], FP, tag="qin")
                    nc.sync.dma_start(
                        out=qb_t,
                        in_=q_in[qb * QB:(qb + 1) * QB, :].rearrange(
                            "(t p) d -> p t d", p=P))
                    qT_blk = blocks.tile([P, DC, QB], FP, tag="qT_blk")
                    transpose_block(qb_t, 0, qT_blk)
                    for uc in range(DC):
                        ps = pps.tile([P, QB], FP, tag="proj")
                        for dc in range(DC):
                            nc.tensor.matmul(
                                ps, wq_sb[:, dc, uc * P:(uc + 1) * P].bitcast(FR),
                                qT_blk[:, dc, :].bitcast(FR),
                                start=(dc == 0), stop=(dc == DC - 1))
                        # qT_p = -(Q^T): (ps + bq) * -1
                        nc.vector.tensor_scalar(
                            qT_p[:, uc, qb * QB:(qb + 1) * QB], ps,
                            bq_sb[:, uc:uc + 1], -1.0, ADD, MUL)

                for kb in range(S // QB):
                    nc.sync.dma_start(
                        out=v_nat[:, kb * TPB:(kb + 1) * TPB, :],
                        in_=v_in[kb * QB:(kb + 1) * QB, :].rearrange(
                            "(t p) d -> p t d", p=P))
                    if kb < NQB:
                        do_qb(kb)
                    vT_blk = blocks.tile([P, DC, QB], FP, tag="vT_blk")
                    transpose_block(v_nat, kb * TPB, vT_blk)

                    # kh = 2 sin^2(pi frac((K@Wr+br)/2pi)) (cos folded into
                    # negated Q via softmax shift invariance)
                    for rc in range(DC):
                        ps = pps.tile([P, QB], FP, tag="proj")
                        for dc in range(DC):
                            nc.tensor.matmul(
                                ps, wkr_sb[:, dc, rc * P:(rc + 1) * P].bitcast(FR),
                                vT_blk[:, dc, :].bitcast(FR),
                                start=(dc == 0), stop=False)
                        nc.tensor.matmul(
                            ps, bkr_row[0:1, rc * P:(rc + 1) * P].bitcast(FR),
                            ones_row.bitcast(FR), start=False, stop=True)
                        sl = slice(kb * QB, (kb + 1) * QB)
                        m_t = khtmp.tile([P, QB], FP, tag="kh_m")
                        nc.gpsimd.tensor_scalar(m_t, ps, MAGIC, MAGIC, ADD, SUB)
                        f_t = khtmp.tile([P, QB], FP, tag="kh_f")
                        nc.gpsimd.tensor_sub(f_t, ps, m_t)
                        s_t = khtmp.tile([P, QB], FP, tag="kh_s")
                        nc.scalar.activation(s_t, f_t, AF.Sin,
                                             scale=float(np.pi))
                        nc.scalar.activation(kh_sb[:, rc, sl], s_t, AF.Square,
                                             scale=SQRT2)

            # ---- stage D: attention ------------------------------------
            with tc.tile_pool(name="attn", bufs=3) as attn, \
                 tc.tile_pool(name="accp", bufs=2) as accp, \
                 tc.tile_pool(name="zsbp", bufs=2) as zsbp, \
                 tc.tile_pool(name="outp", bufs=2) as outp, \
                 tc.tile_pool(name="sc_ps", bufs=2, space="PSUM") as scp, \
                 tc.tile_pool(name="z_ps", bufs=2, space="PSUM") as zp, \
                 tc.tile_pool(name="tr_ps2", bufs=2, space="PSUM") as trp:

                def make_drain(qb, z_banks, acc0, acc1):
                    """Emit-later closure draining q-block qb's accumulators."""
                    state = {}

                    def piece0():
                        # PSUM Z -> SBUF as soon as the Z group stops
                        zsb = zsbp.tile([P, DC, QB], FP, tag="zsb", name="zsb")
                        for dc in range(DC):
                            nc.vector.tensor_copy(zsb[:, dc, :], z_banks[dc])
                        state["zsb"] = zsb

                    def piece1():
                        zsb = state["zsb"]
                        o2 = []
                        for uc in range(DC):
                            o2t = zp.tile([P, QB], FP, tag="o2", bufs=2,
                                          name=f"o2_{uc}")
                            for dc in range(DC):
                                nc.tensor.matmul(
                                    o2t,
                                    wv_sb[:, dc, uc * P:(uc + 1) * P].bitcast(FR),
                                    zsb[:, dc, :].bitcast(FR),
                                    start=(dc == 0), stop=(dc == DC - 1))
                            o2.append(o2t)
                        state["o2"] = o2
                        state["o_sb"] = outp.tile([P, TPB, DIM], FP,
                                                  tag="o_sb", name="o_sb")

                    def piece2(qt):
                        o2 = state["o2"]
                        o_sb = state["o_sb"]
                        qsl = slice(qt * P, (qt + 1) * P)
                        rs_t = trp.tile([P, P], FP, tag="ot_ps")
                        rs = rs_t[:, 0:1]
                        nc.tensor.matmul(rs, acc0[:, qsl], ones_col,
                                         start=True, stop=False)
                        nc.tensor.matmul(rs, acc1[:, qsl], ones_col,
                                         start=False, stop=True)
                        recip = outp.tile([P, 1], FP, tag="recip", bufs=4)
                        nc.vector.reciprocal(recip, rs)
                        for uh in range(DC):
                            ot = outp.tile([P, P], FR, tag="ot", bufs=2)
                            nc.vector.tensor_scalar_add(
                                ot, o2[uh][:, qsl], bv_sb[:, uh:uh + 1])
                            tp = trp.tile([P, P], FR, tag="ot_ps", name="tp")
                            nc.tensor.matmul(tp, ot, ident, is_transpose=True,
                                             start=True, stop=True)
                            nc.vector.tensor_scalar_mul(
                                o_sb[:, qt, uh * P:(uh + 1) * P],
                                tp.bitcast(FP), recip[:])

                    def piece3():
                        row0 = qb * QB
                        nc.sync.dma_start(
                            out=out[row0:row0 + QB, :].rearrange(
                                "(t p) u -> p t u", p=P),
                            in_=state["o_sb"])

                    return piece0, piece1, piece2, piece3

                pending = None
                for qb in range(NQB):
                    qs = slice(qb * QB, (qb + 1) * QB)
                    if pending is not None:
                        pending[0]()
                    z_banks = [zp.tile([P, QB], FP, tag="z", name=f"z{dc}")
                               for dc in range(DC)]
                    acc0 = accp.tile([P, QB], FP, tag="acc0")
                    acc1 = accp.tile([P, QB], FP, tag="acc1")
                    prev = None
                    for kt in range(KT):
                        sc = scp.tile([P, QB], FP, tag="sc")
                        for rc in range(DC):
                            nc.tensor.matmul(
                                sc, kh_sb[:, rc, kt * P:(kt + 1) * P].bitcast(FR),
                                qT_p[:, rc, qs].bitcast(FR),
                                start=(rc == 0), stop=(rc == DC - 1))
                        probs = attn.tile([P, QB], FP, tag="probs")
                        nc.scalar.activation(probs, sc, AF.Exp)
                        if kt == 0:
                            nc.vector.tensor_copy(acc0, probs)
                        elif kt == 1:
                            nc.gpsimd.tensor_copy(acc1, probs)
                        elif kt % 2 == 0:
                            nc.vector.tensor_add(acc0, acc0, probs)
                        else:
                            nc.gpsimd.tensor_add(acc1, acc1, probs)
                        if prev is not None:
                            pk, pp = prev
                            for dc in range(DC):
                                nc.tensor.matmul(
                                    z_banks[dc],
                                    v_nat[:, pk, dc * P:(dc + 1) * P].bitcast(FR),
                                    pp.bitcast(FR),
                                    start=(pk == 0), stop=False)
                        prev = (kt, probs)
                        if pending is not None:
                            if kt == 1:
                                pending[1]()
                            elif kt == 4:
                                pending[2](0)
                            elif kt == 5:
                                pending[2](1)
                            elif kt == 6:
                                pending[2](2)
                            elif kt == 7:
                                pending[2](3)
                                pending[3]()
                                pending = None
                    pk, pp = prev
                    for dc in range(DC):
                        nc.tensor.matmul(
                            z_banks[dc],
                            v_nat[:, pk, dc * P:(dc + 1) * P].bitcast(FR),
                            pp.bitcast(FR), start=False, stop=True)
                    pending = make_drain(qb, z_banks, acc0, acc1)
                # drain the last q-block
                pending[0]()
                pending[1]()
                for qt in range(TPB):
                    pending[2](qt)
                pending[3]()
    nc.finalize()
    return nc


_NC_CACHE = None


def _get_nc():
    global _NC_CACHE
    if _NC_CACHE is None:
        _NC_CACHE = build_kernel(bacc.Bacc(None, target_bir_lowering=False))
    return _NC_CACHE


def kernel(**inputs) -> np.ndarray:
    query = np.ascontiguousarray(np.asarray(inputs["query"], dtype=np.float32))
    value = np.ascontiguousarray(np.asarray(inputs["value"], dtype=np.float32))
    ws = {k: np.ascontiguousarray(np.asarray(inputs[k], dtype=np.float32))
          for k in ("Wq", "bq", "Wk", "bk", "Wv", "bv", "Wr", "br")}
    nc = _get_nc()
    in_maps = []
    for c in range(NC):
        b, h = c // 2, c % 2
        in_maps.append({
            "q_shard": np.ascontiguousarray(query[b, h * SQ:(h + 1) * SQ]),
            "v_full": value[b],
            **ws,
        })
    res = run_bass_kernel_spmd(nc, in_maps, core_ids=list(range(NC)))
    out = np.empty((B, S, DIM), np.float32)
    for c in range(NC):
        b, h = c // 2, c % 2
        out[b, h * SQ:(h + 1) * SQ] = res.results[c]["out"]
    return out
